# revision 1
# baseline (speedup 1.0000x reference)
"""DealerGraphSAGE (2-layer SAGEConv, mean aggregation) on 8 Trainium2 NeuronCores.

Self-contained kernel: takes full inputs, shards internally, returns full output.

Sharding: nodes are range-partitioned across the 8 cores (12500 each); edges are
bucketed by destination core. Per core, local nodes are degree-sorted and a
padded CSR ([128, k_w] slot windows, pads pointing at an all-zero table row) is
built on the host. On device, each CSR slot column is fetched with a 128-row
indirect DMA gather, windows are segment-summed with strided DVE reduces (a ones
column appended to x yields the degrees), the dense SAGE algebra runs on the PE
(h^T = relu(W1_l^T mean^T + W1_r^T x^T + b1)), t = h @ W2_l is stored in rank
(degree-sorted) order and AllGathered across cores, and layer 2 repeats the
gather/reduce against the t table with rank-remapped indices:
z = agg2/deg + h @ W2_r + b2. The host unpermutes z back to natural order.
"""
import numpy as np
from contextlib import ExitStack

import concourse.bacc as bacc
import concourse.tile as tile
import concourse.bass as bass
import concourse.mybir as mybir
from concourse.masks import make_identity

N_NODES = 100000
N_CORES = 8
LOCAL = N_NODES // N_CORES          # 12500
WINP = 128                          # nodes per window (partition dim)
NWIN = (LOCAL + WINP - 1) // WINP   # 98
LOCAL_PAD = NWIN * WINP             # 12544
F_IN, H, OUT = 9, 64, 32
D1 = F_IN + 1                       # features + ones column
XROWS = N_NODES + 1                 # zero row at index 100000
TROWS = N_CORES * LOCAL_PAD
CHUNK = 512


def host_prep(x, edge_index):
    src = np.asarray(edge_index[0], dtype=np.int64)
    dst = np.asarray(edge_index[1], dtype=np.int64)
    x = np.asarray(x, dtype=np.float32)

    x_ext = np.zeros((XROWS, D1), dtype=np.float32)
    x_ext[:N_NODES, :F_IN] = x
    x_ext[:N_NODES, F_IN] = 1.0

    core_of = dst // LOCAL
    per_core = []
    rank_of_all = np.empty(N_NODES, dtype=np.int64)  # global node -> rank in owner core
    for c in range(N_CORES):
        sel = np.nonzero(core_of == c)[0]
        s = src[sel]
        d = dst[sel] - c * LOCAL
        deg = np.bincount(d, minlength=LOCAL)
        order = np.argsort(-deg, kind="stable")       # rank -> local id
        rank_of = np.empty(LOCAL, dtype=np.int64)
        rank_of[order] = np.arange(LOCAL)
        rank_of_all[c * LOCAL:(c + 1) * LOCAL] = rank_of
        eo = np.argsort(rank_of[d], kind="stable")    # edges grouped by dst rank
        s_sorted = s[eo]
        deg_ranked = deg[order]
        offs = np.zeros(LOCAL + 1, dtype=np.int64)
        np.cumsum(deg_ranked, out=offs[1:])
        per_core.append({"order": order, "offs": offs, "s_sorted": s_sorted,
                         "deg_ranked": deg_ranked})

    k_w = np.zeros(NWIN, dtype=np.int64)
    for c in range(N_CORES):
        dr = np.zeros(LOCAL_PAD, dtype=np.int64)
        dr[:LOCAL] = per_core[c]["deg_ranked"]
        np.maximum(k_w, dr.reshape(NWIN, WINP).max(axis=1), out=k_w)
    k_w = np.maximum(k_w, 1)
    offs_w = np.zeros(NWIN + 1, dtype=np.int64)
    np.cumsum(k_w, out=offs_w[1:])
    K_TOT = int(offs_w[-1])

    for c in range(N_CORES):
        pc = per_core[c]
        csr1 = np.full((WINP, K_TOT), N_NODES, dtype=np.int32)
        csr2 = np.full((WINP, K_TOT), LOCAL, dtype=np.int32)  # pad -> zero row (core0 block, row 12500)
        offs, s_sorted = pc["offs"], pc["s_sorted"]
        # t-table row of global src s: owner_core * LOCAL_PAD + rank_in_owner
        s2 = (s_sorted // LOCAL) * LOCAL_PAD + rank_of_all[s_sorted]
        for w in range(NWIN):
            base = int(offs_w[w])
            for p in range(WINP):
                r = w * WINP + p
                if r >= LOCAL:
                    continue
                e0, e1 = int(offs[r]), int(offs[r + 1])
                csr1[p, base:base + (e1 - e0)] = s_sorted[e0:e1]
                csr2[p, base:base + (e1 - e0)] = s2[e0:e1]
        order = pc["order"]
        self_idx = np.full((WINP, NWIN), N_NODES, dtype=np.int32)
        for w in range(NWIN):
            ranks = np.arange(w * WINP, (w + 1) * WINP)
            real = ranks < LOCAL
            self_idx[real, w] = (c * LOCAL + order[ranks[real]]).astype(np.int32)
        pc["csr1"], pc["csr2"] = csr1, csr2
        pc["self_idx"] = self_idx

    return {"x_ext": x_ext, "k_w": k_w, "offs_w": offs_w, "K_TOT": K_TOT}, per_core


def build_program(K_TOT, k_w, offs_w, amp=1):
    """Build the SPMD Bass program. amp>1 wraps the compute body in a hardware
    loop (timing amplification only — results identical each iteration)."""
    dt = mybir.dt
    nc = bacc.Bacc("TRN2", target_bir_lowering=False, debug=False,
                   num_devices=N_CORES)

    x_ext = nc.dram_tensor("x_ext", [XROWS, D1], dt.float32, kind="ExternalInput").ap()
    csr1 = nc.dram_tensor("csr1", [WINP, K_TOT], dt.int32, kind="ExternalInput").ap()
    csr2 = nc.dram_tensor("csr2", [WINP, K_TOT], dt.int32, kind="ExternalInput").ap()
    self_idx = nc.dram_tensor("self_idx", [WINP, NWIN], dt.int32, kind="ExternalInput").ap()
    w1l = nc.dram_tensor("w1l", [F_IN, H], dt.float32, kind="ExternalInput").ap()
    w1r = nc.dram_tensor("w1r", [F_IN, H], dt.float32, kind="ExternalInput").ap()
    b1 = nc.dram_tensor("b1", [H], dt.float32, kind="ExternalInput").ap()
    w2l = nc.dram_tensor("w2l", [H, OUT], dt.float32, kind="ExternalInput").ap()
    w2rb = nc.dram_tensor("w2rb", [H + 1, OUT], dt.float32, kind="ExternalInput").ap()
    z_out = nc.dram_tensor("z_out", [LOCAL_PAD, OUT], dt.float32, kind="ExternalOutput").ap()
    t_local = nc.dram_tensor("t_local", [LOCAL_PAD, OUT], dt.float32).ap()
    t_ext = nc.dram_tensor("t_ext", [TROWS, OUT], dt.float32).ap()

    f32 = dt.float32
    AX = mybir.AxisListType
    OPS = mybir.AluOpType
    ACT = mybir.ActivationFunctionType
    KMAX = int(k_w.max())

    with tile.TileContext(nc) as tc, ExitStack() as ctx:
        pool = ctx.enter_context(tc.tile_pool(name="main", bufs=1))
        gpool = ctx.enter_context(tc.tile_pool(name="gather", bufs=3))
        tpool = ctx.enter_context(tc.tile_pool(name="tchunk", bufs=2))
        pp_t = ctx.enter_context(tc.tile_pool(name="pp_t", bufs=2, space="PSUM"))
        pp_h = ctx.enter_context(tc.tile_pool(name="pp_h", bufs=2, space="PSUM"))
        pp_w = ctx.enter_context(tc.tile_pool(name="pp_w", bufs=1, space="PSUM"))

        ident = pool.tile([128, 128], f32)
        make_identity(nc, ident[:])
        w1l_sb = pool.tile([F_IN, H], f32)
        nc.sync.dma_start(out=w1l_sb[:], in_=w1l[:])
        w1r_sb = pool.tile([F_IN, H], f32)
        nc.sync.dma_start(out=w1r_sb[:], in_=w1r[:])
        b1_sb = pool.tile([H, 1], f32)
        nc.sync.dma_start(out=b1_sb[:], in_=b1[:, None])
        w2l_sb = pool.tile([H, OUT], f32)
        nc.sync.dma_start(out=w2l_sb[:], in_=w2l[:])
        w2rb_sb = pool.tile([H + 1, OUT], f32)
        nc.sync.dma_start(out=w2rb_sb[:], in_=w2rb[:])

        csr1_sb = pool.tile([WINP, K_TOT], dt.int32)
        nc.sync.dma_start(out=csr1_sb[:], in_=csr1[:])
        csr2_sb = pool.tile([WINP, K_TOT], dt.int32)
        nc.sync.dma_start(out=csr2_sb[:], in_=csr2[:])
        self_sb = pool.tile([WINP, NWIN], dt.int32)
        nc.sync.dma_start(out=self_sb[:], in_=self_idx[:])

        zrow = pool.tile([1, OUT], f32)
        nc.vector.memset(zrow[:], 0.0)

        agg1 = pool.tile([WINP, NWIN, D1], f32)
        xs = pool.tile([WINP, NWIN, D1], f32)
        recip = pool.tile([WINP, NWIN, 1], f32)
        meanS = pool.tile([WINP, NWIN, F_IN], f32)
        hT = pool.tile([H + 1, LOCAL_PAD], f32)
        tNP = pool.tile([WINP, NWIN, OUT], f32)
        agg2 = pool.tile([WINP, NWIN, OUT], f32)
        zNP = pool.tile([WINP, NWIN, OUT], f32)

        def body_pre(_i=None):
            # ---- layer-1 gather + segmented reduce ----
            for w in range(NWIN):
                kw = int(k_w[w]); base = int(offs_w[w])
                g = gpool.tile([WINP, KMAX, D1], f32, tag="g1")
                for j in range(kw):
                    nc.gpsimd.indirect_dma_start(
                        out=g[:, j, :], out_offset=None, in_=x_ext[:],
                        in_offset=bass.IndirectOffsetOnAxis(
                            ap=csr1_sb[:, base + j:base + j + 1], axis=0))
                nc.vector.tensor_reduce(
                    out=agg1[:, w, :], in_=g[:, 0:kw, :].rearrange("p j d -> p d j"),
                    axis=AX.X, op=OPS.add)
            for w in range(NWIN):
                nc.gpsimd.indirect_dma_start(
                    out=xs[:, w, :], out_offset=None, in_=x_ext[:],
                    in_offset=bass.IndirectOffsetOnAxis(ap=self_sb[:, w:w + 1], axis=0))

            nc.vector.tensor_scalar(out=recip[:], in0=agg1[:, :, F_IN:D1],
                                    scalar1=1.0, scalar2=None, op0=OPS.max)
            nc.vector.reciprocal(out=recip[:], in_=recip[:])
            nc.vector.tensor_tensor(out=meanS[:], in0=agg1[:, :, 0:F_IN],
                                    in1=recip[:].to_broadcast([WINP, NWIN, F_IN]),
                                    op=OPS.mult)

            # ---- h^T = relu(W1_l^T mean^T + W1_r^T x^T + b1); row H = ones ----
            nc.vector.memset(hT[H:H + 1, :], 1.0)
            WPC = CHUNK // WINP
            n_chunks = (NWIN + WPC - 1) // WPC
            for ci in range(n_chunks):
                w0 = ci * WPC
                wn = min(WPC, NWIN - w0)
                cols = wn * WINP
                mT = tpool.tile([F_IN, CHUNK], f32, tag="mT")
                xT = tpool.tile([F_IN, CHUNK], f32, tag="xT")
                for wi in range(wn):
                    ps_m = pp_t.tile([F_IN, WINP], f32, tag="tp9")
                    nc.tensor.transpose(out=ps_m[:], in_=meanS[:, w0 + wi, :], identity=ident[:])
                    nc.scalar.activation(out=mT[:, wi * WINP:(wi + 1) * WINP], in_=ps_m[:], func=ACT.Copy)
                    ps_x = pp_t.tile([F_IN, WINP], f32, tag="tp9")
                    nc.tensor.transpose(out=ps_x[:], in_=xs[:, w0 + wi, 0:F_IN], identity=ident[:])
                    nc.scalar.activation(out=xT[:, wi * WINP:(wi + 1) * WINP], in_=ps_x[:], func=ACT.Copy)
                ps_h = pp_h.tile([H, CHUNK], f32, tag="ph")
                nc.tensor.matmul(out=ps_h[:, 0:cols], lhsT=w1l_sb[:], rhs=mT[:, 0:cols], start=True, stop=False)
                nc.tensor.matmul(out=ps_h[:, 0:cols], lhsT=w1r_sb[:], rhs=xT[:, 0:cols], start=False, stop=True)
                nc.scalar.activation(out=hT[0:H, w0 * WINP:w0 * WINP + cols], in_=ps_h[:, 0:cols],
                                     func=ACT.Relu, bias=b1_sb[:])

            # ---- t = h @ W2_l, stored in rank order (static writes) ----
            for ci in range(n_chunks):
                w0 = ci * WPC
                wn = min(WPC, NWIN - w0)
                cols = wn * WINP
                ps_tc = pp_h.tile([OUT, CHUNK], f32, tag="pt")
                nc.tensor.matmul(out=ps_tc[:, 0:cols], lhsT=w2l_sb[:],
                                 rhs=hT[0:H, w0 * WINP:w0 * WINP + cols], start=True, stop=True)
                tTc = tpool.tile([OUT, CHUNK], f32, tag="tTc")
                nc.scalar.activation(out=tTc[:, 0:cols], in_=ps_tc[:, 0:cols], func=ACT.Copy)
                for wi in range(wn):
                    ps_w = pp_w.tile([WINP, OUT], f32, tag="tw")
                    nc.tensor.transpose(out=ps_w[:], in_=tTc[:, wi * WINP:(wi + 1) * WINP],
                                        identity=ident[0:OUT, 0:OUT])
                    nc.vector.tensor_copy(out=tNP[:, w0 + wi, :], in_=ps_w[:])
            for w in range(NWIN):
                nc.sync.dma_start(out=t_local[w * WINP:(w + 1) * WINP, :], in_=tNP[:, w, :])
            # rank rows >= LOCAL hold relu(b1)-derived junk; row LOCAL is the
            # global zero row referenced by layer-2 CSR pads
            nc.sync.dma_start(out=t_local[LOCAL:LOCAL + 1, :], in_=zrow[:])

        def collective():
            nc.gpsimd.collective_compute(
                "AllGather", OPS.bypass,
                replica_groups=[list(range(N_CORES))],
                ins=[t_local[:]], outs=[t_ext[:]])

        def body_post(_i=None):
            # ---- layer-2 gather + reduce ----
            for w in range(NWIN):
                kw = int(k_w[w]); base = int(offs_w[w])
                g = gpool.tile([WINP, KMAX, OUT], f32, tag="g2")
                for j in range(kw):
                    nc.gpsimd.indirect_dma_start(
                        out=g[:, j, :], out_offset=None, in_=t_ext[:],
                        in_offset=bass.IndirectOffsetOnAxis(
                            ap=csr2_sb[:, base + j:base + j + 1], axis=0))
                nc.vector.tensor_reduce(
                    out=agg2[:, w, :], in_=g[:, 0:kw, :].rearrange("p j d -> p d j"),
                    axis=AX.X, op=OPS.add)

            # ---- z = agg2/deg + h @ W2_r + b2 (rank order, static writes) ----
            for w in range(NWIN):
                ps_zr = pp_w.tile([WINP, OUT], f32, tag="zw")
                nc.tensor.matmul(out=ps_zr[:], lhsT=hT[:, w * WINP:(w + 1) * WINP],
                                 rhs=w2rb_sb[:], start=True, stop=True)
                nc.vector.tensor_tensor(out=zNP[:, w, :], in0=agg2[:, w, :],
                                        in1=recip[:, w, :].to_broadcast([WINP, OUT]),
                                        op=OPS.mult)
                nc.vector.tensor_tensor(out=zNP[:, w, :], in0=zNP[:, w, :],
                                        in1=ps_zr[:], op=OPS.add)
            for w in range(NWIN):
                nc.sync.dma_start(out=z_out[w * WINP:(w + 1) * WINP, :], in_=zNP[:, w, :])

        if amp == 1:
            body_pre()
            collective()
            body_post()
        else:
            with tc.For_i(0, amp, 1) as i:
                body_pre(i)
            collective()
            with tc.For_i(0, amp, 1) as i:
                body_post(i)

    nc.compile()
    return nc


_CACHE = {}


def get_program(x, edge_index, amp=1):
    shared, per_core = host_prep(x, edge_index)
    key = (shared["K_TOT"], amp)
    if key not in _CACHE:
        _CACHE[key] = build_program(shared["K_TOT"], shared["k_w"], shared["offs_w"], amp=amp)
    return _CACHE[key], shared, per_core


def make_in_maps(shared, per_core, W1_l, b1, W1_r, W2_l, b2, W2_r):
    w2rb = np.concatenate([np.asarray(W2_r, np.float32),
                           np.asarray(b2, np.float32)[None, :]], axis=0)
    in_maps = []
    for c in range(N_CORES):
        pc = per_core[c]
        in_maps.append({
            "x_ext": shared["x_ext"],
            "csr1": pc["csr1"], "csr2": pc["csr2"], "self_idx": pc["self_idx"],
            "w1l": np.asarray(W1_l, np.float32), "w1r": np.asarray(W1_r, np.float32),
            "b1": np.asarray(b1, np.float32),
            "w2l": np.asarray(W2_l, np.float32), "w2rb": w2rb,
        })
    return in_maps


def assemble_output(outs, per_core):
    z = np.empty((N_NODES, OUT), dtype=np.float32)
    for c in range(N_CORES):
        zc = outs[c]["z_out"][:LOCAL]            # rank order
        z[c * LOCAL + per_core[c]["order"]] = zc  # rank r -> natural node order[r]
    return z


def kernel(x, edge_index, W1_l, b1, W1_r, W2_l, b2, W2_r):
    nc, shared, per_core = get_program(x, edge_index)
    in_maps = make_in_maps(shared, per_core, W1_l, b1, W1_r, W2_l, b2, W2_r)
    from concourse.bass_utils import run_bass_kernel_spmd
    res = run_bass_kernel_spmd(nc, in_maps, list(range(N_CORES)))
    return assemble_output(res.results, per_core)



# revision 3
# speedup vs baseline: 1.4214x; 1.4214x over previous
"""DealerGraphSAGE (2-layer SAGEConv, mean aggregation) on 8 Trainium2 NeuronCores.

Self-contained kernel: takes full inputs, shards internally, returns full output.

Sharding: nodes are range-partitioned across the 8 cores (12500 each); edges are
bucketed by destination core. Per core, local nodes are degree-sorted and a
padded CSR ([128, k_w] slot windows, pads pointing at an all-zero table row) is
built on the host. On device, each CSR slot column is fetched with a 128-row
indirect DMA gather, windows are segment-summed with strided DVE reduces (a ones
column appended to x yields the degrees), the dense SAGE algebra runs on the PE
(h^T = relu(W1_l^T mean^T + W1_r^T x^T + b1)), t = h @ W2_l is stored in rank
(degree-sorted) order and AllGathered across cores, and layer 2 repeats the
gather/reduce against the t table with rank-remapped indices:
z = agg2/deg + h @ W2_r + b2. The host unpermutes z back to natural order.
"""
import numpy as np
from contextlib import ExitStack

import concourse.bacc as bacc
import concourse.tile as tile
import concourse.bass as bass
import concourse.mybir as mybir
from concourse.masks import make_identity

N_NODES = 100000
N_CORES = 8
LOCAL = N_NODES // N_CORES          # 12500
WINP = 128                          # nodes per window (partition dim)
NWIN = (LOCAL + WINP - 1) // WINP   # 98
LOCAL_PAD = NWIN * WINP             # 12544
F_IN, H, OUT = 9, 64, 32
D1 = F_IN + 1                       # features + ones column
XROWS = N_NODES + 1                 # zero row at index 100000
TROWS = N_CORES * LOCAL_PAD
CHUNK = 512


def host_prep(x, edge_index):
    src = np.asarray(edge_index[0], dtype=np.int64)
    dst = np.asarray(edge_index[1], dtype=np.int64)
    x = np.asarray(x, dtype=np.float32)


    core_of = dst // LOCAL
    per_core = []
    rank_of_all = np.empty(N_NODES, dtype=np.int64)  # global node -> rank in owner core
    for c in range(N_CORES):
        sel = np.nonzero(core_of == c)[0]
        s = src[sel]
        d = dst[sel] - c * LOCAL
        deg = np.bincount(d, minlength=LOCAL)
        order = np.argsort(-deg, kind="stable")       # rank -> local id
        rank_of = np.empty(LOCAL, dtype=np.int64)
        rank_of[order] = np.arange(LOCAL)
        rank_of_all[c * LOCAL:(c + 1) * LOCAL] = rank_of
        eo = np.argsort(rank_of[d], kind="stable")    # edges grouped by dst rank
        s_sorted = s[eo]
        deg_ranked = deg[order]
        offs = np.zeros(LOCAL + 1, dtype=np.int64)
        np.cumsum(deg_ranked, out=offs[1:])
        per_core.append({"order": order, "offs": offs, "s_sorted": s_sorted,
                         "deg_ranked": deg_ranked})

    k_w = np.zeros(NWIN, dtype=np.int64)
    for c in range(N_CORES):
        dr = np.zeros(LOCAL_PAD, dtype=np.int64)
        dr[:LOCAL] = per_core[c]["deg_ranked"]
        np.maximum(k_w, dr.reshape(NWIN, WINP).max(axis=1), out=k_w)
    k_w = np.maximum(k_w, 1)
    offs_w = np.zeros(NWIN + 1, dtype=np.int64)
    np.cumsum(k_w, out=offs_w[1:])
    K_TOT = int(offs_w[-1])

    # rank-ordered x table: row (c*LOCAL_PAD + rank) = x of that node; pad rows
    # (rank >= LOCAL) are zero; row 12500 doubles as the CSR pad target
    x_rank = np.zeros((TROWS, D1), dtype=np.float32)
    for c in range(N_CORES):
        order = per_core[c]["order"]
        x_rank[c * LOCAL_PAD:c * LOCAL_PAD + LOCAL, :F_IN] = x[c * LOCAL + order]
        x_rank[c * LOCAL_PAD:c * LOCAL_PAD + LOCAL, F_IN] = 1.0

    for c in range(N_CORES):
        pc = per_core[c]
        csr2 = np.full((WINP, K_TOT), LOCAL, dtype=np.int32)  # pad -> zero row (core0 block, row 12500)
        offs, s_sorted = pc["offs"], pc["s_sorted"]
        # rank-space row of global src s: owner_core * LOCAL_PAD + rank_in_owner
        s2 = (s_sorted // LOCAL) * LOCAL_PAD + rank_of_all[s_sorted]
        for w in range(NWIN):
            base = int(offs_w[w])
            for p in range(WINP):
                r = w * WINP + p
                if r >= LOCAL:
                    continue
                e0, e1 = int(offs[r]), int(offs[r + 1])
                csr2[p, base:base + (e1 - e0)] = s2[e0:e1]
        pc["csr2"] = csr2
        pc["xself"] = np.ascontiguousarray(x_rank[c * LOCAL_PAD:(c + 1) * LOCAL_PAD])

    return {"x_ext": x_rank, "k_w": k_w, "offs_w": offs_w, "K_TOT": K_TOT}, per_core


def build_program(K_TOT, k_w, offs_w, amp=1):
    """Build the SPMD Bass program. amp>1 wraps the compute body in a hardware
    loop (timing amplification only — results identical each iteration)."""
    dt = mybir.dt
    nc = bacc.Bacc("TRN2", target_bir_lowering=False, debug=False,
                   num_devices=N_CORES)

    x_ext = nc.dram_tensor("x_ext", [TROWS, D1], dt.float32, kind="ExternalInput").ap()
    csr2 = nc.dram_tensor("csr2", [WINP, K_TOT], dt.int32, kind="ExternalInput").ap()
    xself = nc.dram_tensor("xself", [LOCAL_PAD, D1], dt.float32, kind="ExternalInput").ap()
    w1l = nc.dram_tensor("w1l", [F_IN, H], dt.float32, kind="ExternalInput").ap()
    w1r = nc.dram_tensor("w1r", [F_IN, H], dt.float32, kind="ExternalInput").ap()
    b1 = nc.dram_tensor("b1", [H], dt.float32, kind="ExternalInput").ap()
    w2l = nc.dram_tensor("w2l", [H, OUT], dt.float32, kind="ExternalInput").ap()
    w2rb = nc.dram_tensor("w2rb", [H + 1, OUT], dt.float32, kind="ExternalInput").ap()
    z_out = nc.dram_tensor("z_out", [LOCAL_PAD, OUT], dt.float32, kind="ExternalOutput").ap()
    t_local = nc.dram_tensor("t_local", [LOCAL_PAD, OUT], dt.float32).ap()
    t_ext = nc.dram_tensor("t_ext", [TROWS, OUT], dt.float32).ap()

    f32 = dt.float32
    AX = mybir.AxisListType
    OPS = mybir.AluOpType
    ACT = mybir.ActivationFunctionType
    KMAX = int(k_w.max())

    with tile.TileContext(nc) as tc, ExitStack() as ctx:
        pool = ctx.enter_context(tc.tile_pool(name="main", bufs=1))
        gpool = ctx.enter_context(tc.tile_pool(name="gather", bufs=3))
        tpool = ctx.enter_context(tc.tile_pool(name="tchunk", bufs=2))
        pp_t = ctx.enter_context(tc.tile_pool(name="pp_t", bufs=2, space="PSUM"))
        pp_h = ctx.enter_context(tc.tile_pool(name="pp_h", bufs=2, space="PSUM"))
        pp_w = ctx.enter_context(tc.tile_pool(name="pp_w", bufs=1, space="PSUM"))

        ident = pool.tile([128, 128], f32)
        make_identity(nc, ident[:])
        w1l_sb = pool.tile([F_IN, H], f32)
        nc.sync.dma_start(out=w1l_sb[:], in_=w1l[:])
        w1r_sb = pool.tile([F_IN, H], f32)
        nc.sync.dma_start(out=w1r_sb[:], in_=w1r[:])
        b1_sb = pool.tile([H, 1], f32)
        nc.sync.dma_start(out=b1_sb[:], in_=b1[:, None])
        w2l_sb = pool.tile([H, OUT], f32)
        nc.sync.dma_start(out=w2l_sb[:], in_=w2l[:])
        w2rb_sb = pool.tile([H + 1, OUT], f32)
        nc.sync.dma_start(out=w2rb_sb[:], in_=w2rb[:])

        csr2_sb = pool.tile([WINP, K_TOT], dt.int32)
        nc.sync.dma_start(out=csr2_sb[:], in_=csr2[:])

        zrow = pool.tile([1, OUT], f32)
        nc.vector.memset(zrow[:], 0.0)

        agg1 = pool.tile([WINP, NWIN, D1], f32)
        xs = pool.tile([WINP, NWIN, D1], f32)
        recip = pool.tile([WINP, NWIN, 1], f32)
        meanS = pool.tile([WINP, NWIN, F_IN], f32)
        hT = pool.tile([H + 1, LOCAL_PAD], f32)
        tNP = pool.tile([WINP, NWIN, OUT], f32)
        agg2 = pool.tile([WINP, NWIN, OUT], f32)
        zNP = pool.tile([WINP, NWIN, OUT], f32)

        def body_pre(_i=None):
            # ---- layer-1 gather + segmented reduce ----
            for w in range(NWIN):
                kw = int(k_w[w]); base = int(offs_w[w])
                g = gpool.tile([WINP, KMAX, D1], f32, tag="g1")
                for j in range(kw):
                    nc.gpsimd.indirect_dma_start(
                        out=g[:, j, :], out_offset=None, in_=x_ext[:],
                        in_offset=bass.IndirectOffsetOnAxis(
                            ap=csr2_sb[:, base + j:base + j + 1], axis=0))
                nc.vector.tensor_reduce(
                    out=agg1[:, w, :], in_=g[:, 0:kw, :].rearrange("p j d -> p d j"),
                    axis=AX.X, op=OPS.add)
            nc.sync.dma_start(
                out=xs[:], in_=xself[:].rearrange("(w p) d -> p w d", p=WINP))

            nc.vector.tensor_scalar(out=recip[:], in0=agg1[:, :, F_IN:D1],
                                    scalar1=1.0, scalar2=None, op0=OPS.max)
            nc.vector.reciprocal(out=recip[:], in_=recip[:])
            nc.vector.tensor_tensor(out=meanS[:], in0=agg1[:, :, 0:F_IN],
                                    in1=recip[:].to_broadcast([WINP, NWIN, F_IN]),
                                    op=OPS.mult)

            # ---- h^T = relu(W1_l^T mean^T + W1_r^T x^T + b1); row H = ones ----
            nc.vector.memset(hT[H:H + 1, :], 1.0)
            WPC = CHUNK // WINP
            n_chunks = (NWIN + WPC - 1) // WPC
            for ci in range(n_chunks):
                w0 = ci * WPC
                wn = min(WPC, NWIN - w0)
                cols = wn * WINP
                mT = tpool.tile([F_IN, CHUNK], f32, tag="mT")
                xT = tpool.tile([F_IN, CHUNK], f32, tag="xT")
                for wi in range(wn):
                    ps_m = pp_t.tile([F_IN, WINP], f32, tag="tp9")
                    nc.tensor.transpose(out=ps_m[:], in_=meanS[:, w0 + wi, :], identity=ident[:])
                    nc.scalar.activation(out=mT[:, wi * WINP:(wi + 1) * WINP], in_=ps_m[:], func=ACT.Copy)
                    ps_x = pp_t.tile([F_IN, WINP], f32, tag="tp9")
                    nc.tensor.transpose(out=ps_x[:], in_=xs[:, w0 + wi, 0:F_IN], identity=ident[:])
                    nc.scalar.activation(out=xT[:, wi * WINP:(wi + 1) * WINP], in_=ps_x[:], func=ACT.Copy)
                ps_h = pp_h.tile([H, CHUNK], f32, tag="ph")
                nc.tensor.matmul(out=ps_h[:, 0:cols], lhsT=w1l_sb[:], rhs=mT[:, 0:cols], start=True, stop=False)
                nc.tensor.matmul(out=ps_h[:, 0:cols], lhsT=w1r_sb[:], rhs=xT[:, 0:cols], start=False, stop=True)
                nc.scalar.activation(out=hT[0:H, w0 * WINP:w0 * WINP + cols], in_=ps_h[:, 0:cols],
                                     func=ACT.Relu, bias=b1_sb[:])

            # ---- t = h @ W2_l, stored in rank order (static writes) ----
            for ci in range(n_chunks):
                w0 = ci * WPC
                wn = min(WPC, NWIN - w0)
                cols = wn * WINP
                ps_tc = pp_h.tile([OUT, CHUNK], f32, tag="pt")
                nc.tensor.matmul(out=ps_tc[:, 0:cols], lhsT=w2l_sb[:],
                                 rhs=hT[0:H, w0 * WINP:w0 * WINP + cols], start=True, stop=True)
                tTc = tpool.tile([OUT, CHUNK], f32, tag="tTc")
                nc.scalar.activation(out=tTc[:, 0:cols], in_=ps_tc[:, 0:cols], func=ACT.Copy)
                for wi in range(wn):
                    ps_w = pp_w.tile([WINP, OUT], f32, tag="tw")
                    nc.tensor.transpose(out=ps_w[:], in_=tTc[:, wi * WINP:(wi + 1) * WINP],
                                        identity=ident[0:OUT, 0:OUT])
                    nc.vector.tensor_copy(out=tNP[:, w0 + wi, :], in_=ps_w[:])
            for w in range(NWIN):
                nc.sync.dma_start(out=t_local[w * WINP:(w + 1) * WINP, :], in_=tNP[:, w, :])
            # rank rows >= LOCAL hold relu(b1)-derived junk; row LOCAL is the
            # global zero row referenced by layer-2 CSR pads
            nc.sync.dma_start(out=t_local[LOCAL:LOCAL + 1, :], in_=zrow[:])

        def collective():
            nc.gpsimd.collective_compute(
                "AllGather", OPS.bypass,
                replica_groups=[list(range(N_CORES))],
                ins=[t_local[:]], outs=[t_ext[:]])

        def body_post(_i=None):
            # ---- layer-2 gather + reduce ----
            for w in range(NWIN):
                kw = int(k_w[w]); base = int(offs_w[w])
                g = gpool.tile([WINP, KMAX, OUT], f32, tag="g2")
                for j in range(kw):
                    nc.gpsimd.indirect_dma_start(
                        out=g[:, j, :], out_offset=None, in_=t_ext[:],
                        in_offset=bass.IndirectOffsetOnAxis(
                            ap=csr2_sb[:, base + j:base + j + 1], axis=0))
                nc.vector.tensor_reduce(
                    out=agg2[:, w, :], in_=g[:, 0:kw, :].rearrange("p j d -> p d j"),
                    axis=AX.X, op=OPS.add)

            # ---- z = agg2/deg + h @ W2_r + b2 (rank order, static writes) ----
            for w in range(NWIN):
                ps_zr = pp_w.tile([WINP, OUT], f32, tag="zw")
                nc.tensor.matmul(out=ps_zr[:], lhsT=hT[:, w * WINP:(w + 1) * WINP],
                                 rhs=w2rb_sb[:], start=True, stop=True)
                nc.vector.tensor_tensor(out=zNP[:, w, :], in0=agg2[:, w, :],
                                        in1=recip[:, w, :].to_broadcast([WINP, OUT]),
                                        op=OPS.mult)
                nc.vector.tensor_tensor(out=zNP[:, w, :], in0=zNP[:, w, :],
                                        in1=ps_zr[:], op=OPS.add)
            for w in range(NWIN):
                nc.sync.dma_start(out=z_out[w * WINP:(w + 1) * WINP, :], in_=zNP[:, w, :])

        if amp == 1:
            body_pre()
            collective()
            body_post()
        else:
            with tc.For_i(0, amp, 1) as i:
                body_pre(i)
            collective()
            with tc.For_i(0, amp, 1) as i:
                body_post(i)

    nc.compile()
    return nc


_CACHE = {}


def get_program(x, edge_index, amp=1):
    shared, per_core = host_prep(x, edge_index)
    key = (shared["K_TOT"], amp)
    if key not in _CACHE:
        _CACHE[key] = build_program(shared["K_TOT"], shared["k_w"], shared["offs_w"], amp=amp)
    return _CACHE[key], shared, per_core


def make_in_maps(shared, per_core, W1_l, b1, W1_r, W2_l, b2, W2_r):
    w2rb = np.concatenate([np.asarray(W2_r, np.float32),
                           np.asarray(b2, np.float32)[None, :]], axis=0)
    in_maps = []
    for c in range(N_CORES):
        pc = per_core[c]
        in_maps.append({
            "x_ext": shared["x_ext"],
            "csr2": pc["csr2"], "xself": pc["xself"],
            "w1l": np.asarray(W1_l, np.float32), "w1r": np.asarray(W1_r, np.float32),
            "b1": np.asarray(b1, np.float32),
            "w2l": np.asarray(W2_l, np.float32), "w2rb": w2rb,
        })
    return in_maps


def assemble_output(outs, per_core):
    z = np.empty((N_NODES, OUT), dtype=np.float32)
    for c in range(N_CORES):
        zc = outs[c]["z_out"][:LOCAL]            # rank order
        z[c * LOCAL + per_core[c]["order"]] = zc  # rank r -> natural node order[r]
    return z


def kernel(x, edge_index, W1_l, b1, W1_r, W2_l, b2, W2_r):
    nc, shared, per_core = get_program(x, edge_index)
    in_maps = make_in_maps(shared, per_core, W1_l, b1, W1_r, W2_l, b2, W2_r)
    from concourse.bass_utils import run_bass_kernel_spmd
    res = run_bass_kernel_spmd(nc, in_maps, list(range(N_CORES)))
    return assemble_output(res.results, per_core)

